# revision 12
# baseline (speedup 1.0000x reference)
"""DeepVCP Trainium kernel.

Split of work:
- Host (numpy): everything derived from *coordinates only* — farthest point
  sampling, ball-query neighbor selection, 3-NN indices/weights, parameter
  folding, layout marshaling.
- Device (Bass/Tile, 8 NeuronCores, SPMD): all feature computation — the
  neighborhood gathers, the pointnet++ set-abstraction MLPs + max-pools,
  feature-propagation interpolation + MLPs, final fc, and the DFE MLPs.
  Launch 1 (L1): one core per (batch, cloud) pointnet2 -> per-point features.
  Launch 2 (L2): one core per batch: DFE branches -> dfs / dft.

All float32, mirroring the reference's per-site arithmetic to ulp level.
"""
import numpy as np
from contextlib import ExitStack

import concourse.bass as bass
import concourse.tile as tile
from concourse import bacc, mybir
from concourse import bass_utils

F32 = mybir.dt.float32
I16 = mybir.dt.int16
AF = mybir.ActivationFunctionType
ALU = mybir.AluOpType

B, N = 2, 8192
NP1, NP2, NP3 = 4096, 1024, 512
NS = 32
TOPK, KDFE, GRID_R, VOXN = 64, 32, 0.4, 2
NCORES = 8
CH = 4096  # streaming chunk (columns)

# ----------------------------------------------------------------- host math

def _fold_bn(layer):
    W, b, g, be = [np.asarray(x, np.float32) for x in layer]
    return (W * g[:, None]).astype(np.float32), (b * g + be).astype(np.float32)


def _lhsT(W, b):
    return np.concatenate([W, b[:, None]], 1).T.astype(np.float32).copy()


def _fps_batch(xyz, npoint):
    U, Nn, _ = xyz.shape
    dist = np.full((U, Nn), 1e10, np.float32)
    far = np.zeros(U, np.int64)
    out = np.empty((U, npoint), np.int64)
    ar = np.arange(U)
    x, y, z = xyz[..., 0], xyz[..., 1], xyz[..., 2]
    for t in range(npoint):
        out[:, t] = far
        c = xyz[ar, far]
        dx = x - c[:, 0:1]; dy = y - c[:, 1:2]; dz = z - c[:, 2:3]
        d = (dx * dx + dy * dy) + dz * dz
        np.minimum(dist, d, out=dist)
        far = np.argmax(dist, 1)
    return out


def _sqdist(q, s):
    q = q.astype(np.float32); s = s.astype(np.float32)
    q2 = np.einsum('qc,qc->q', q, q).astype(np.float32)
    s2 = np.einsum('sc,sc->s', s, s).astype(np.float32)
    return ((q2[:, None] + s2[None, :]) - 2.0 * (q @ s.T)).astype(np.float32)


def _ball_query(r2, ns, q, s):
    Nn = s.shape[0]
    d = _sqdist(q, s)
    scores = np.where(d <= r2, np.arange(Nn, dtype=np.int64)[None, :], Nn)
    part = np.partition(scores, ns - 1, axis=1)[:, :ns]
    part.sort(axis=1)
    first = part[:, 0:1]
    idx = np.where(part == Nn, first, part)
    return np.minimum(idx, Nn - 1).astype(np.int64)


def _three_nn(q, s):
    d = _sqdist(q, s)
    take = np.argpartition(d, 2, axis=1)[:, :3]
    dt = np.take_along_axis(d, take, 1)
    order = np.argsort(dt, axis=1, kind="stable")
    idx = np.take_along_axis(take, order, 1)
    dt = np.take_along_axis(dt, order, 1)
    w = (1.0 / (dt + np.float32(1e-8))).astype(np.float32)
    w = (w / w.sum(1, keepdims=True)).astype(np.float32)
    return idx.astype(np.int64), w


def _wrap_idx(idx_flat, bands):
    J = idx_flat.shape[0]
    assert J % 16 == 0
    w = idx_flat.reshape(J // 16, 16).T.astype(np.int16)
    return np.tile(w, (bands, 1)).copy()


def _pad_k4(idx, w):
    Q = idx.shape[0]
    i4 = np.zeros((Q, 4), np.int64); i4[:, :3] = idx
    w4 = np.zeros((Q, 4), np.float32); w4[:, :3] = w
    return i4.reshape(-1), w4.reshape(-1)


# ------------------------------------------------------------- device builders

_NC_CACHE = {}
_SKIP_PRELOAD = ("gx1", "gx2", "gx3", "gxA", "gxB", "nn3w", "nn2w", "nn1w",
                 "idx1w", "idx2w", "idx3w", "nn3i", "nn2i", "nn1i", "g1src")


def _gather(nc, gsrc, idxs_sl, C, Nn, out_sl, jw):
    nc.gpsimd.ap_gather(out_sl, gsrc, idxs_sl, channels=C, num_elems=Nn, d=1, num_idxs=jw)


def _layer(nc, sbuf, psum, pieces, lts, M, J, out, r0, relu=True, pool_ns=None, tag=""):
    """out = act(sum_i lts[i].T @ pieces[i]) streamed in 512-col chunks.
    pieces: list of (tile, row0, K, col0) — cols aligned with the J range.
    pool_ns: grouped max over pool_ns then relu -> out cols J//pool_ns."""
    for j0 in range(0, J, 512):
        jw = min(512, J - j0)
        ps = psum.tile([M, 512], F32, tag="pst")
        np_ = len(pieces)
        for i, ((t, rr0, K, c0), lt) in enumerate(zip(pieces, lts)):
            nc.tensor.matmul(ps[:, :jw], lt[:], t[rr0:rr0 + K, c0 + j0:c0 + j0 + jw],
                             start=(i == 0), stop=(i == np_ - 1))
        if pool_ns:
            q0, qw = j0 // pool_ns, jw // pool_ns
            red = sbuf.tile([M, 512 // pool_ns], F32, tag="redt")
            nc.vector.tensor_reduce(red[:, :qw], ps[:, :jw].rearrange("m (q s) -> m q s", s=pool_ns),
                                    axis=mybir.AxisListType.X, op=ALU.max)
            nc.scalar.activation(out[r0:r0 + M, q0:q0 + qw], red[:, :qw], AF.Relu)
        elif relu:
            nc.scalar.activation(out[r0:r0 + M, j0:j0 + jw], ps[:, :jw], AF.Relu)
        else:
            nc.scalar.copy(out[r0:r0 + M, j0:j0 + jw], ps[:, :jw])


def _build_l1():
    nc = bacc.Bacc("TRN2", target_bir_lowering=False, debug=False, num_devices=NCORES)
    di = {}
    def inp(name, shape, dt=F32):
        di[name] = nc.dram_tensor(name, shape, dt, kind="ExternalInput").ap()
        return di[name]

    J1, J2, J3 = NP1 * NS, NP2 * NS, NP3 * NS
    Jn3, Jn2, Jn1 = NP2 * 4, NP1 * 4, N * 4
    inp("g1src", [16, N])
    inp("gx1", [3, J1]); inp("idx1w", [16, J1 // 16], I16)
    inp("gx2", [3, J2]); inp("idx2w", [48, J2 // 16], I16)
    inp("gx3", [3, J3]); inp("idx3w", [80, J3 // 16], I16)
    inp("nn3i", [64, Jn3 // 16], I16); inp("nn3w", [1, Jn3])
    inp("nn2i", [64, Jn2 // 16], I16); inp("nn2w", [1, Jn2])
    inp("nn1i", [32, Jn1 // 16], I16); inp("nn1w", [1, Jn1])
    for nm, shape in [("sa1_gx", [3, 32]), ("sa1_ft", [5, 32]), ("sa1_l2", [33, 32]),
                      ("sa2_gx", [3, 32]), ("sa2_ft", [33, 32]), ("sa2_l2", [33, 64]),
                      ("sa3_gx", [3, 64]), ("sa3_ft", [65, 64]), ("sa3_l2", [65, 64]),
                      ("fp3_l1a", [64, 64]), ("fp3_l1b", [65, 64]), ("fp3_l2", [65, 64]),
                      ("fp2_l1a", [32, 32]), ("fp2_l1b", [65, 32]), ("fp2_l2", [33, 32]),
                      ("fp1_l1", [33, 32]), ("fp1_l2", [33, 32]), ("fp1_l3", [33, 32]),
                      ("fc", [33, 32])]:
        inp(nm, shape)
    sf_d = nc.dram_tensor("sf", [32, N], F32, kind="ExternalOutput").ap()

    with tile.TileContext(nc) as tc, ExitStack() as ctx:
        sbuf = ctx.enter_context(tc.tile_pool(name="sb", bufs=1))
        psum = ctx.enter_context(tc.tile_pool(name="ps", bufs=4, space="PSUM"))
        ld = {}
        for nm, ap in di.items():
            if nm in _SKIP_PRELOAD:
                continue
            t = sbuf.tile(list(ap.shape), ap.dtype, tag=f"in_{nm}")
            nc.sync.dma_start(t[:], ap[:])
            ld[nm] = t

        def sa_fixed(gsrcT, cgath, gx_name, idx_name, Q, Nn, names, cout, out, tag):
            J = Q * NS
            l_gx, l_ft, l2 = ld[names[0]], ld[names[1]], ld[names[2]]
            Kft = l_ft.shape[0]
            M1 = l_gx.shape[1]
            K2 = l2.shape[0]
            for j0 in range(0, J, CH):
                jw = min(CH, J - j0)
                idxt = sbuf.tile([cgath, CH // 16], I16, tag="idxbuf")
                nc.sync.dma_start(idxt[:, :jw // 16], di[idx_name][:, j0 // 16:(j0 + jw) // 16])
                g = sbuf.tile([cgath, CH], F32, tag="gbuf")
                _gather(nc, gsrcT, idxt[:, :jw // 16], cgath, Nn, g[:, :jw], jw)
                gx = sbuf.tile([3, CH], F32, tag="gxbuf")
                nc.sync.dma_start(gx[:, :jw], di[gx_name][:, j0:j0 + jw])
                h1 = sbuf.tile([K2, CH], F32, tag="h1buf")
                for m0 in range(0, jw, 512):
                    mw = min(512, jw - m0)
                    ps = psum.tile([M1, 512], F32, tag="pst")
                    nc.tensor.matmul(ps[:, :mw], l_gx[:], gx[:, m0:m0 + mw], start=True, stop=False)
                    nc.tensor.matmul(ps[:, :mw], l_ft[:], g[0:Kft, m0:m0 + mw], start=False, stop=True)
                    nc.scalar.activation(h1[0:M1, m0:m0 + mw], ps[:, :mw], AF.Relu)
                nc.vector.memset(h1[K2 - 1:K2, :jw], 1.0)
                _layer(nc, sbuf, psum, [(h1, 0, K2, 0)], [l2], cout, jw,
                       out[:, j0 // NS:(j0 + jw) // NS], 0, pool_ns=NS, tag=f"l2{tag}")

        def interp(srcT, D, nni, nnw_name, Q, S, out, r0, tag):
            J = Q * 4
            onesd = sbuf.tile([1, 128], F32, tag="onesd")
            nc.vector.memset(onesd[:], 1.0)
            for j0 in range(0, J, CH):
                jw = min(CH, J - j0)
                idxt = sbuf.tile([D, CH // 16], I16, tag="idxbuf")
                nc.sync.dma_start(idxt[:, :jw // 16], di[nni][:, j0 // 16:(j0 + jw) // 16])
                g = sbuf.tile([D, CH], F32, tag="gbuf")
                _gather(nc, srcT[0:D], idxt[:, :jw // 16], D, S, g[:, :jw], jw)
                wrow = sbuf.tile([1, CH], F32, tag="gxbuf")
                nc.sync.dma_start(wrow[:, :jw], di[nnw_name][:, j0:j0 + jw])
                wb = sbuf.tile([D, CH], F32, tag="h1buf")
                for m0 in range(0, jw, 512):
                    mw = min(512, jw - m0)
                    ps = psum.tile([D, 512], F32, tag="pst")
                    nc.tensor.matmul(ps[:, :mw], onesd[:, 0:D], wrow[:, m0:m0 + mw],
                                     start=True, stop=True)
                    nc.vector.tensor_copy(wb[:, m0:m0 + mw], ps[:, :mw])
                nc.vector.tensor_mul(g[0:D, :jw], g[0:D, :jw], wb[:, :jw])
                nc.vector.tensor_reduce(out[r0:r0 + D, j0 // 4:(j0 + jw) // 4],
                                        g[0:D, :jw].rearrange("d (q k) -> d q k", k=4),
                                        axis=mybir.AxisListType.X, op=ALU.add)

        # SA1
        g1s = sbuf.tile([16, N], F32, tag="hx")
        nc.sync.dma_start(g1s[:], di["g1src"][:])
        pts1 = sbuf.tile([48, NP1], F32, tag="pts1")
        sa_fixed(g1s, 16, "gx1", "idx1w", NP1, N,
                 ("sa1_gx", "sa1_ft", "sa1_l2"), 32, pts1, "s1")
        nc.vector.memset(pts1[32:48, :], 0.0)
        nc.vector.memset(pts1[32:33, :], 1.0)
        # SA2
        pts2 = sbuf.tile([80, NP2], F32, tag="pts2")
        sa_fixed(pts1, 48, "gx2", "idx2w", NP2, NP1,
                 ("sa2_gx", "sa2_ft", "sa2_l2"), 64, pts2, "s2")
        nc.vector.memset(pts2[64:80, :], 0.0)
        nc.vector.memset(pts2[64:65, :], 1.0)
        # SA3
        pts3 = sbuf.tile([64, NP3], F32, tag="pts3")
        sa_fixed(pts2, 80, "gx3", "idx3w", NP3, NP2,
                 ("sa3_gx", "sa3_ft", "sa3_l2"), 64, pts3, "s3")
        # FP3
        itp3 = sbuf.tile([65, NP2], F32, tag="itp3")
        nc.vector.memset(itp3[64:65, :], 1.0)
        interp(pts3, 64, "nn3i", "nn3w", NP2, NP3, itp3, 0, "n3")
        pts2b = sbuf.tile([65, NP2], F32, tag="pts2b")
        _layer(nc, sbuf, psum, [(pts2, 0, 64, 0), (itp3, 0, 65, 0)],
               [ld["fp3_l1a"], ld["fp3_l1b"]], 64, NP2, pts2b, 0, tag="f3a")
        nc.vector.memset(pts2b[64:65, :], 1.0)
        pts2c = sbuf.tile([64, NP2], F32, tag="pts2c")
        _layer(nc, sbuf, psum, [(pts2b, 0, 65, 0)], [ld["fp3_l2"]], 64, NP2, pts2c, 0, tag="f3b")
        # FP2
        itp2 = sbuf.tile([65, NP1], F32, tag="fp2_in")
        nc.vector.memset(itp2[64:65, :], 1.0)
        interp(pts2c, 64, "nn2i", "nn2w", NP1, NP2, itp2, 0, "n2")
        pts1b = sbuf.tile([33, NP1], F32, tag="itp3")
        _layer(nc, sbuf, psum, [(pts1, 0, 32, 0), (itp2, 0, 65, 0)],
               [ld["fp2_l1a"], ld["fp2_l1b"]], 32, NP1, pts1b, 0, tag="f2a")
        nc.vector.memset(pts1b[32:33, :], 1.0)
        pts1c = sbuf.tile([33, NP1], F32, tag="pts1c")
        _layer(nc, sbuf, psum, [(pts1b, 0, 33, 0)], [ld["fp2_l2"]], 32, NP1, pts1c, 0, tag="f2b")
        # FP1
        fp1_in = sbuf.tile([33, N], F32, tag="hx")
        nc.vector.memset(fp1_in[32:33, :], 1.0)
        interp(pts1c, 32, "nn1i", "nn1w", N, NP1, fp1_in, 0, "n1")
        ha = sbuf.tile([33, N], F32, tag="hy")
        _layer(nc, sbuf, psum, [(fp1_in, 0, 33, 0)], [ld["fp1_l1"]], 32, N, ha, 0, tag="f1a")
        nc.vector.memset(ha[32:33, :], 1.0)
        hb = sbuf.tile([33, N], F32, tag="hx")
        _layer(nc, sbuf, psum, [(ha, 0, 33, 0)], [ld["fp1_l2"]], 32, N, hb, 0, tag="f1b")
        nc.vector.memset(hb[32:33, :], 1.0)
        hc = sbuf.tile([33, N], F32, tag="hy")
        _layer(nc, sbuf, psum, [(hb, 0, 33, 0)], [ld["fp1_l3"]], 32, N, hc, 0, tag="f1c")
        nc.vector.memset(hc[32:33, :], 1.0)
        sf = sbuf.tile([32, N], F32, tag="hx")
        _layer(nc, sbuf, psum, [(hc, 0, 33, 0)], [ld["fc"]], 32, N, sf, 0, relu=False, tag="fcl")
        nc.sync.dma_start(sf_d[:], sf[:])
    nc.compile()
    return nc


def _build_l2():
    nc = bacc.Bacc("TRN2", target_bir_lowering=False, debug=False, num_devices=NCORES)
    di = {}
    def inp(name, shape, dt=F32):
        di[name] = nc.dram_tensor(name, shape, dt, kind="ExternalInput").ap()
        return di[name]
    JA, JB = TOPK * KDFE, TOPK * (VOXN ** 3) * KDFE
    inp("gAsrc", [48, N]); inp("gxA", [3, JA]); inp("idxAw", [48, JA // 16], I16)
    inp("gBsrc", [48, N]); inp("gxB", [3, JB]); inp("idxBw", [48, JB // 16], I16)
    inp("dfe_gx", [3, 32]); inp("dfe_ft", [34, 32]); inp("dfe_l2", [33, 32]); inp("dfe_l3", [33, 32])
    dfs_d = nc.dram_tensor("dfsP", [32, TOPK], F32, kind="ExternalOutput").ap()
    hB_d = nc.dram_tensor("hB", [32, JB], F32, kind="ExternalOutput").ap()

    with tile.TileContext(nc) as tc, ExitStack() as ctx:
        sbuf = ctx.enter_context(tc.tile_pool(name="sb", bufs=1))
        psum = ctx.enter_context(tc.tile_pool(name="ps", bufs=4, space="PSUM"))
        ld = {}
        for nm, ap in di.items():
            if nm in _SKIP_PRELOAD:
                continue
            t = sbuf.tile(list(ap.shape), ap.dtype, tag=f"in_{nm}")
            nc.sync.dma_start(t[:], ap[:])
            ld[nm] = t

        def dfe(side, J, out_sb, pool, out_dram):
            for j0 in range(0, J, CH):
                jw = min(CH, J - j0)
                g = sbuf.tile([48, CH], F32, tag="gbuf")
                _gather(nc, ld[f"g{side}src"], ld[f"idx{side}w"][:, j0 // 16:(j0 + jw) // 16],
                        48, N, g[:, :jw], jw)
                gx = sbuf.tile([3, CH], F32, tag="gxbuf")
                nc.sync.dma_start(gx[:, :jw], di[f"gx{side}"][:, j0:j0 + jw])
                h1 = sbuf.tile([33, CH], F32, tag="h1buf")
                for m0 in range(0, jw, 512):
                    mw = min(512, jw - m0)
                    ps = psum.tile([32, 512], F32, tag="pst")
                    nc.tensor.matmul(ps[:, :mw], ld["dfe_gx"][:], gx[:, m0:m0 + mw],
                                     start=True, stop=False)
                    nc.tensor.matmul(ps[:, :mw], ld["dfe_ft"][:], g[0:34, m0:m0 + mw],
                                     start=False, stop=True)
                    nc.scalar.activation(h1[0:32, m0:m0 + mw], ps[:, :mw], AF.Relu)
                nc.vector.memset(h1[32:33, :jw], 1.0)
                h2 = sbuf.tile([33, CH], F32, tag="h2buf")
                _layer(nc, sbuf, psum, [(h1, 0, 33, 0)], [ld["dfe_l2"]], 32, jw, h2, 0, tag=f"d2{side}")
                nc.vector.memset(h2[32:33, :jw], 1.0)
                h3 = sbuf.tile([32, CH // (KDFE if pool else 1)], F32, tag="h3buf")
                if pool:
                    _layer(nc, sbuf, psum, [(h2, 0, 33, 0)], [ld["dfe_l3"]], 32, jw, h3, 0,
                           pool_ns=KDFE, tag=f"d3{side}")
                    nc.vector.tensor_copy(out_sb[:, j0 // KDFE:(j0 + jw) // KDFE], h3[:, : jw // KDFE])
                else:
                    _layer(nc, sbuf, psum, [(h2, 0, 33, 0)], [ld["dfe_l3"]], 32, jw, h3, 0, tag=f"d3{side}")
                    nc.sync.dma_start(out_dram[:, j0:j0 + jw], h3[:, :jw])

        dfsP = sbuf.tile([32, TOPK], F32, tag="dfsP")
        dfe("A", JA, dfsP, True, None)
        nc.sync.dma_start(dfs_d[:], dfsP[:])
        dfe("B", JB, None, False, hB_d)
    nc.compile()
    return nc


# ----------------------------------------------------------------- the kernel

def kernel(source, target, T_prev, params):
    source = np.asarray(source, np.float32)
    target = np.asarray(target, np.float32)
    T_prev = np.asarray(T_prev, np.float32)
    pn2, wl, dfe_p = params['pn2'], params['wl'], params['dfe']

    f = {}
    for nm in ("sa1", "sa2", "sa3", "fp3", "fp2", "fp1"):
        f[nm] = [_fold_bn(l) for l in pn2[nm]]
    fcW, fcb = np.asarray(pn2['fc'][0], np.float32), np.asarray(pn2['fc'][1], np.float32)
    dW = [(np.asarray(W, np.float32), np.asarray(bb, np.float32)) for (W, bb) in dfe_p]

    units = [source[0], target[0], source[1], target[1]]
    U = len(units)
    xyz = np.stack([u[:3].T.copy() for u in units])

    fidx1 = _fps_batch(xyz, NP1)
    xyz1 = np.stack([xyz[u][fidx1[u]] for u in range(U)])
    fidx2 = _fps_batch(xyz1, NP2)
    xyz2 = np.stack([xyz1[u][fidx2[u]] for u in range(U)])
    fidx3 = _fps_batch(xyz2, NP3)
    xyz3 = np.stack([xyz2[u][fidx3[u]] for u in range(U)])

    in_maps1 = []
    for u in range(U):
        i1 = _ball_query(0.1 * 0.1, NS, xyz1[u], xyz[u])
        i2 = _ball_query(0.2 * 0.2, NS, xyz2[u], xyz1[u])
        i3 = _ball_query(0.4 * 0.4, NS, xyz3[u], xyz2[u])
        gx1 = (xyz[u][i1.reshape(-1)] - np.repeat(xyz1[u], NS, 0)).T.copy()
        gx2 = (xyz1[u][i2.reshape(-1)] - np.repeat(xyz2[u], NS, 0)).T.copy()
        gx3 = (xyz2[u][i3.reshape(-1)] - np.repeat(xyz3[u], NS, 0)).T.copy()
        n3i, n3w = _three_nn(xyz2[u], xyz3[u])
        n2i, n2w = _three_nn(xyz1[u], xyz2[u])
        n1i, n1w = _three_nn(xyz[u], xyz1[u])
        n3i, n3w = _pad_k4(n3i, n3w)
        n2i, n2w = _pad_k4(n2i, n2w)
        n1i, n1w = _pad_k4(n1i, n1w)
        g1src = np.zeros((16, N), np.float32)
        g1src[0:4] = units[u]
        g1src[4] = 1.0
        m = {
            "g1src": g1src,
            "gx1": gx1, "idx1w": _wrap_idx(i1.reshape(-1), 1),
            "gx2": gx2, "idx2w": _wrap_idx(i2.reshape(-1), 3),
            "gx3": gx3, "idx3w": _wrap_idx(i3.reshape(-1), 5),
            "nn3i": _wrap_idx(n3i, 4), "nn3w": n3w.reshape(1, -1),
            "nn2i": _wrap_idx(n2i, 4), "nn2w": n2w.reshape(1, -1),
            "nn1i": _wrap_idx(n1i, 2), "nn1w": n1w.reshape(1, -1),
        }
        (W1, b1), (W2, b2) = f["sa1"]
        m["sa1_gx"] = W1[:, :3].T.copy(); m["sa1_ft"] = _lhsT(W1[:, 3:7], b1)
        m["sa1_l2"] = _lhsT(W2, b2)
        (W1, b1), (W2, b2) = f["sa2"]
        m["sa2_gx"] = W1[:, :3].T.copy(); m["sa2_ft"] = _lhsT(W1[:, 3:35], b1)
        m["sa2_l2"] = _lhsT(W2, b2)
        (W1, b1), (W2, b2) = f["sa3"]
        m["sa3_gx"] = W1[:, :3].T.copy(); m["sa3_ft"] = _lhsT(W1[:, 3:67], b1)
        m["sa3_l2"] = _lhsT(W2, b2)
        (W, bb) = f["fp3"][0]
        m["fp3_l1a"] = W[:, :64].T.copy(); m["fp3_l1b"] = _lhsT(W[:, 64:128], bb)
        m["fp3_l2"] = _lhsT(*f["fp3"][1])
        (W, bb) = f["fp2"][0]
        m["fp2_l1a"] = W[:, :32].T.copy(); m["fp2_l1b"] = _lhsT(W[:, 32:96], bb)
        m["fp2_l2"] = _lhsT(*f["fp2"][1])
        for li, (W, bb) in enumerate(f["fp1"]):
            m[f"fp1_l{li + 1}"] = _lhsT(W, bb)
        m["fc"] = _lhsT(fcW, fcb)
        in_maps1.append(m)

    nc1 = _NC_CACHE.get("l1")
    if nc1 is None:
        nc1 = _NC_CACHE["l1"] = _build_l1()
    res1 = bass_utils.run_bass_kernel_spmd(nc1, in_maps1 + in_maps1[:NCORES - U],
                                           core_ids=list(range(NCORES)))
    sf_all = [res1.results[u]["sf"] for u in range(U)]

    (W1, b1, g1, be1), (W2, b2, g2, be2), (W3, b3) = [
        tuple(np.asarray(x, np.float32) for x in layer) for layer in wl]
    in_maps2 = []
    lin = np.linspace(-GRID_R, GRID_R, VOXN).astype(np.float32)
    grid = np.stack(np.meshgrid(lin, lin, lin, indexing='ij'), -1).reshape(-1, 3).astype(np.float32)
    for b in range(B):
        sf = sf_all[2 * b]
        tf = sf_all[2 * b + 1]
        h = np.maximum((W1 @ sf + b1[:, None]) * g1[None, :] + be1[None, :], 0)
        h = np.maximum((W2 @ h + b2[:, None]) * g2[None, :] + be2[None, :], 0)
        z = (W3 @ h + b3[:, None])[0]
        topk = np.argsort(-z, kind="stable")[:TOPK]
        keypts = source[b][:, topk].T.copy()
        kp = keypts[:, :3].astype(np.float32)
        iA = _ball_query(1.0, KDFE, kp, sf[:3].T.copy())
        gxA = (sf[:3].T[iA.reshape(-1)] - np.repeat(kp, KDFE, 0)).T.copy()
        tkp = (keypts @ T_prev[b]).astype(np.float32)
        cand = (grid[None, :, :] + tkp[:, None, :3]).reshape(-1, 3).astype(np.float32)
        iB = _ball_query(1.0, KDFE, cand, tf[:3].T.copy())
        gxB = (tf[:3].T[iB.reshape(-1)] - np.repeat(cand, KDFE, 0)).T.copy()
        def gsrc_of(feat, pcl):
            gs = np.zeros((48, N), np.float32)
            gs[0] = pcl[3]
            gs[1:33] = feat
            gs[33] = 1.0
            return gs
        m = {
            "gAsrc": gsrc_of(sf, source[b]), "gxA": gxA,
            "idxAw": _wrap_idx(iA.reshape(-1), 3),
            "gBsrc": gsrc_of(tf, target[b]), "gxB": gxB,
            "idxBw": _wrap_idx(iB.reshape(-1), 3),
            "dfe_gx": dW[0][0][:, :3].T.copy(),
            "dfe_ft": _lhsT(dW[0][0][:, 3:36], dW[0][1]),
            "dfe_l2": _lhsT(*dW[1]), "dfe_l3": _lhsT(*dW[2]),
        }
        in_maps2.append(m)
    nc2 = _NC_CACHE.get("l2")
    if nc2 is None:
        nc2 = _NC_CACHE["l2"] = _build_l2()
    res2 = bass_utils.run_bass_kernel_spmd(nc2, in_maps2 * (NCORES // B),
                                           core_ids=list(range(NCORES)))

    dfs = np.stack([res2.results[b]["dfsP"].T for b in range(B)])
    dft = np.stack([res2.results[b]["hB"].max(0).reshape(TOPK, VOXN ** 3, KDFE, 1)
                    for b in range(B)])
    src_out = np.transpose(source, (0, 2, 1)).copy()
    return src_out, dfs, dft
